# revision 14
# baseline (speedup 1.0000x reference)
"""CPM3 cross-attention on 8 Trainium2 NeuronCores.

Sharding: 2-way data-parallel over batch x 4-way tensor-parallel over heads
(8 heads per core). Each core computes a partial output (its heads' slice of
the out-projection, pre-residual); the host sums the 4 partials per batch and
adds the residual.

Per-core math (fp16 operands on all projection/score matmuls, fp32 PSUM):
  rsqrt-scale of RMSNorm is applied to Q after the projection (linearity);
  ln_weight is folded into w_q on the host. Scores are computed transposed
  (S.T[k, t]) so no on-device transposes are needed. Softmax uses a fixed
  -60 offset instead of a row max (safe for this distribution) and the
  position bias + mask are applied MULTIPLICATIVELY after the exp:
  P = exp(S - 60) * expb where expb = mask * exp(position_bias) is
  precomputed on the host (bf16). Z = sum_k P via DVE pair-sums + short
  ones-matmul chains; normalization (1/Z) uses the fast approx reciprocal
  and is applied after P@V. V stays resident in SBUF (no DRAM spill).
"""

import numpy as np
import ml_dtypes
from contextlib import ExitStack

B = 2
T = 1024          # LQ == LK
D = 4096
H = 32
DH = 128
G = 8             # heads per core
ND = D // 128     # 32 contraction tiles
EPS = 1e-5
COFF = 60.0       # softmax fixed offset

_PROGRAM = None
LAST_RESULT = None


def _build_program():
    import concourse.mybir as mybir
    import concourse.tile as tile
    from concourse import bacc

    dt = mybir.dt
    f32 = dt.float32
    f32r = dt.float32r
    bf16 = dt.bfloat16
    f16 = dt.float16
    AF = mybir.ActivationFunctionType

    nc = bacc.Bacc(None, target_bir_lowering=False, dynamic_dma_scratch_size=2048)

    xT_d = nc.dram_tensor("xT", [D, T], f16, kind="ExternalInput")
    kvT_d = nc.dram_tensor("kvT", [D, T], f16, kind="ExternalInput")
    wqT_d = nc.dram_tensor("wqT", [D, G * DH], f16, kind="ExternalInput")
    wkT_d = nc.dram_tensor("wkT", [D, G * DH], f16, kind="ExternalInput")
    wvT_d = nc.dram_tensor("wvT", [D, G * DH], f16, kind="ExternalInput")
    woT_d = nc.dram_tensor("woT", [G * DH, D], f16, kind="ExternalInput")
    ebT_d = nc.dram_tensor("ebT", [G, T, T], bf16, kind="ExternalInput")
    onc_d = nc.dram_tensor("onc", [128, 1], f32r, kind="ExternalInput")
    onk_d = nc.dram_tensor("onk", [1, 128], f32r, kind="ExternalInput")
    oncb_d = nc.dram_tensor("oncb", [128, 1], bf16, kind="ExternalInput")
    out_d = nc.dram_tensor("out", [T, D], f32, kind="ExternalOutput")

    with (
        tile.TileContext(nc) as tc,
        ExitStack() as ctx,
        nc.allow_low_precision(reason="fp16/bf16 attention pipeline"),
    ):
        consts = ctx.enter_context(tc.tile_pool(name="consts", bufs=1))
        ones_k1 = consts.tile([1, 128], f32r)
        nc.sync.dma_start(out=ones_k1, in_=onk_d[:, :])
        ones_colb = consts.tile([128, 1], bf16)
        nc.sync.dma_start(out=ones_colb, in_=oncb_d[:, :])
        mbias = consts.tile([128, 1], f32)
        nc.vector.memset(mbias, -COFF)
        ebias = consts.tile([1, 1], f32)
        nc.vector.memset(ebias, EPS)

        kt_pool = ctx.enter_context(tc.tile_pool(name="kt", bufs=1))
        kt_sb = kt_pool.tile([128, G, T], f16)   # K.T: [dh(part), head, k]
        v_pool = ctx.enter_context(tc.tile_pool(name="v", bufs=1))
        v_sb = v_pool.tile([128, 8, G * DH], bf16)  # V: [k%128(part), kb, o]
        qt_pool = ctx.enter_context(tc.tile_pool(name="qt", bufs=1))
        qt_sb = qt_pool.tile([128, G, T], f16)  # Q.T: [dh(part), head, t]

        with tc.tile_pool(name="x", bufs=1) as x_pool:
            x_sb = x_pool.tile([128, ND, T], f16)

            # ---------------- Phase KV: K.T and V projections ----------------
            with (
                tc.tile_pool(name="kv", bufs=1) as kv_pool,
                tc.tile_pool(name="wkv", bufs=3) as w_pool,
                tc.tile_pool(name="pskv", bufs=1, space="PSUM") as ps_pool,
            ):
                kv_sb = kv_pool.tile([128, ND, T], f16)
                # first chunk split fine so the round-0 d=0 matmuls start ASAP
                for i in range(4):
                    nc.scalar.dma_start(
                        out=kv_sb[:, i : i + 1, :],
                        in_=kvT_d.rearrange("(n p) t -> p n t", p=128)[
                            :, i : i + 1, :
                        ],
                    )
                for i in range(1, 8):
                    nc.scalar.dma_start(
                        out=kv_sb[:, i * 4 : (i + 1) * 4, :],
                        in_=kvT_d.rearrange("(n p) t -> p n t", p=128)[
                            :, i * 4 : (i + 1) * 4, :
                        ],
                    )
                # K.T: 2 rounds over o-halves, 8 psum banks (DMA-paced round 0)
                for r in range(2):
                    if r == 1:
                        # prefetch x for the Q phase; issued after round-0 DMAs
                        # so the critical kv/wk transfers get the early bandwidth
                        for i in range(8):
                            nc.scalar.dma_start(
                                out=x_sb[:, i * 4 : (i + 1) * 4, :],
                                in_=xT_d.rearrange("(n p) t -> p n t", p=128)[
                                    :, i * 4 : (i + 1) * 4, :
                                ],
                            )
                    acc = [
                        ps_pool.tile([128, 512], f32, tag="acc", bufs=8, name=f"acck{r}_{j}")
                        for j in range(8)
                    ]
                    for dg in range(ND // 4):
                        wk_t = w_pool.tile([128, 4, 512], f16, tag="wk")
                        nc.sync.dma_start(
                            out=wk_t,
                            in_=wkT_d.rearrange("(n p) o -> p n o", p=128)[
                                :, dg * 4 : (dg + 1) * 4, r * 512 : (r + 1) * 512
                            ],
                        )
                        for dd in range(4):
                            d = dg * 4 + dd
                            for ob2 in range(4):
                                for tkb in range(2):
                                    nc.tensor.matmul(
                                        acc[ob2 * 2 + tkb],
                                        wk_t[:, dd, ob2 * 128 : (ob2 + 1) * 128],
                                        kv_sb[:, d, tkb * 512 : (tkb + 1) * 512],
                                        start=(d == 0),
                                        stop=(d == ND - 1),
                                    )
                    for ob2 in range(4):
                        for tkb in range(2):
                            ob = r * 4 + ob2
                            dst = kt_sb[:, ob, tkb * 512 : (tkb + 1) * 512]
                            src = acc[ob2 * 2 + tkb]
                            if tkb == 0:
                                nc.vector.tensor_copy(out=dst, in_=src)
                            else:
                                nc.scalar.activation(
                                    out=dst, in_=src, func=AF.Copy, bias=0.0, scale=1.0
                                )

                # V (fp16 matmuls from resident kv, resident bf16 output)
                for r in range(2):
                    acc8 = [
                        ps_pool.tile([128, 512], f32, tag="acc", bufs=8, name=f"accv{r}_{j}")
                        for j in range(8)
                    ]
                    for dg in range(ND // 4):
                        wv_t = w_pool.tile([128, 4, 512], f16, tag="wv")
                        nc.sync.dma_start(
                            out=wv_t,
                            in_=wvT_d.rearrange("(n p) o -> p n o", p=128)[
                                :, dg * 4 : (dg + 1) * 4, r * 512 : (r + 1) * 512
                            ],
                        )
                        for dd in range(4):
                            d = dg * 4 + dd
                            for tkb in range(8):
                                nc.tensor.matmul(
                                    acc8[tkb],
                                    kv_sb[:, d, tkb * 128 : (tkb + 1) * 128],
                                    wv_t[:, dd, :],
                                    start=(d == 0),
                                    stop=(d == ND - 1),
                                )
                    for tkb in range(8):
                        dst = v_sb[:, tkb, r * 512 : (r + 1) * 512]
                        if tkb % 2 == 0:
                            nc.vector.tensor_copy(out=dst, in_=acc8[tkb])
                        else:
                            nc.scalar.activation(
                                out=dst, in_=acc8[tkb], func=AF.Copy, bias=0.0, scale=1.0
                            )

            # ---------------- Phase Q: rsqrt stats + Q projection ----------------
            with (
                tc.tile_pool(name="wq", bufs=3) as wq_pool,
                tc.tile_pool(name="sq", bufs=2) as sq_pool,
                tc.tile_pool(name="srow", bufs=1) as srow_pool,
                tc.tile_pool(name="psq", bufs=1, space="PSUM") as ps_pool,
            ):
                # var[t] = sum_d x^2 (ACT Square + ones-matmul), interleaved with
                # Q-proj round 0.
                var_ps = [
                    ps_pool.tile([1, 512], f32, tag="var", bufs=2, name=f"var{j}")
                    for j in range(2)
                ]
                acc0 = [
                    ps_pool.tile([128, 512], f32, tag="acc", bufs=6, name=f"accq0_{j}")
                    for j in range(4)
                ]
                for dg in range(ND // 4):
                    wq_t = wq_pool.tile([128, 4, 256], f16, tag="wqt")
                    nc.sync.dma_start(
                        out=wq_t,
                        in_=wqT_d.rearrange("(n p) o -> p n o", p=128)[
                            :, dg * 4 : (dg + 1) * 4, 0:256
                        ],
                    )
                    sqs = [[], []]
                    for dd in range(4):
                        d = dg * 4 + dd
                        for tb in range(2):
                            sq = sq_pool.tile([128, 512], bf16, tag="sq", bufs=12)
                            nc.scalar.activation(
                                out=sq,
                                in_=x_sb[:, d, tb * 512 : (tb + 1) * 512],
                                func=AF.Square,
                            )
                            sqs[tb].append(sq)
                        for ob2 in range(2):
                            for tb in range(2):
                                nc.tensor.matmul(
                                    acc0[ob2 * 2 + tb],
                                    wq_t[:, dd, ob2 * 128 : (ob2 + 1) * 128],
                                    x_sb[:, d, tb * 512 : (tb + 1) * 512],
                                    start=(d == 0),
                                    stop=(d == ND - 1),
                                )
                    # quad-sum the squares on DVE so var costs 1 matmul per dg
                    for tb in range(2):
                        qa = sq_pool.tile([128, 512], bf16, tag="sqa", bufs=8)
                        nc.vector.tensor_add(out=qa, in0=sqs[tb][0], in1=sqs[tb][1])
                        qb = sq_pool.tile([128, 512], bf16, tag="sqa", bufs=8)
                        nc.vector.tensor_add(out=qb, in0=sqs[tb][2], in1=sqs[tb][3])
                        qd = sq_pool.tile([128, 512], bf16, tag="sqa", bufs=8)
                        nc.vector.tensor_add(out=qd, in0=qa, in1=qb)
                        nc.tensor.matmul(
                            var_ps[tb], ones_colb, qd,
                            start=(dg == 0), stop=(dg == ND // 4 - 1),
                        )
                # s = 1/sqrt(var/D + eps), replicated to 128 partitions
                # replicate sqrt(var/D+eps) to 128 partitions, then take the
                # approx reciprocal wide (128 lanes) instead of on one row
                sd_row = srow_pool.tile([1, T], f32r)
                for tb in range(2):
                    nc.scalar.activation(
                        out=sd_row[:, tb * 512 : (tb + 1) * 512],
                        in_=var_ps[tb],
                        func=AF.Sqrt,
                        bias=ebias,
                        scale=1.0 / D,
                    )
                s_rep = srow_pool.tile([128, T], f32)
                for tb in range(2):
                    srp = ps_pool.tile([128, 512], f32, tag="var", bufs=2, name=f"srp{tb}")
                    nc.tensor.matmul(
                        srp, ones_k1,
                        sd_row[:, tb * 512 : (tb + 1) * 512],
                        start=True, stop=True,
                    )
                    nc.vector.reciprocal_approx_fast(
                        out=s_rep[:, tb * 512 : (tb + 1) * 512], in_=srp
                    )

                # Q.T round-0 writes (scale folded), then rounds 1-3
                for ob2 in range(2):
                    for tb in range(2):
                        nc.vector.tensor_mul(
                            out=qt_sb[:, ob2, tb * 512 : (tb + 1) * 512],
                            in0=acc0[ob2 * 2 + tb],
                            in1=s_rep[:, tb * 512 : (tb + 1) * 512],
                        )
                for r in range(1, 4):
                    acc = [
                        ps_pool.tile([128, 512], f32, tag="acc", bufs=6, name=f"accq{r}_{j}")
                        for j in range(4)
                    ]
                    for dg in range(ND // 4):
                        wq_t = wq_pool.tile([128, 4, 256], f16, tag="wqt")
                        nc.sync.dma_start(
                            out=wq_t,
                            in_=wqT_d.rearrange("(n p) o -> p n o", p=128)[
                                :, dg * 4 : (dg + 1) * 4, r * 256 : (r + 1) * 256
                            ],
                        )
                        for dd in range(4):
                            d = dg * 4 + dd
                            for ob2 in range(2):
                                for tb in range(2):
                                    nc.tensor.matmul(
                                        acc[ob2 * 2 + tb],
                                        wq_t[:, dd, ob2 * 128 : (ob2 + 1) * 128],
                                        x_sb[:, d, tb * 512 : (tb + 1) * 512],
                                        start=(d == 0),
                                        stop=(d == ND - 1),
                                    )
                    for ob2 in range(2):
                        for tb in range(2):
                            ob = r * 2 + ob2
                            nc.vector.tensor_mul(
                                out=qt_sb[:, ob, tb * 512 : (tb + 1) * 512],
                                in0=acc[ob2 * 2 + tb],
                                in1=s_rep[:, tb * 512 : (tb + 1) * 512],
                            )

        at_pool = ctx.enter_context(tc.tile_pool(name="at", bufs=1))
        at_sb = at_pool.tile([128, G, T], f16)  # A_all.T: [dh(part), head, t]

        # ---------------- Phase attention (per head) ----------------
        with (
            tc.tile_pool(name="ebp", bufs=3) as eb_pool,
            tc.tile_pool(name="esp", bufs=6) as es_pool,
            tc.tile_pool(name="pp", bufs=20) as p_pool,
            tc.tile_pool(name="trp", bufs=8) as tr_pool,
            tc.tile_pool(name="zrow", bufs=4) as z_pool,
            tc.tile_pool(name="psat", bufs=1, space="PSUM") as ps_pool,
        ):
            def emit_scores(h, expb_t):
                """Scores + exp + bias-multiply for head h; P tiles [128, 1024]."""
                p_tiles = []
                for kb in range(8):
                    sps = ps_pool.tile([128, 1024], f32, tag="sc", bufs=2,
                                       name=f"sc{h}_{kb}")
                    for tb in range(2):
                        nc.tensor.matmul(
                            sps[:, tb * 512 : (tb + 1) * 512],
                            kt_sb[:, h, kb * 128 : (kb + 1) * 128],
                            qt_sb[:, h, tb * 512 : (tb + 1) * 512],
                            start=True,
                            stop=True,
                        )
                    es = es_pool.tile([128, 1024], bf16, tag="es", name=f"es{h}_{kb}")
                    nc.scalar.activation(
                        out=es, in_=sps, func=AF.Exp, bias=mbias, scale=1.0
                    )
                    p_t = p_pool.tile([128, 1024], bf16, tag="p", name=f"p{h}_{kb}")
                    nc.vector.tensor_mul(out=p_t, in0=es, in1=expb_t[:, kb, :])
                    p_tiles.append(p_t)
                return p_tiles

            def emit_reduce(h, p_tiles):
                """Z, AV, normalize for head h."""
                prs = []
                for j in range(4):
                    pr = tr_pool.tile([128, 1024], bf16, tag="tr", bufs=12, name=f"tr{h}_{j}")
                    nc.vector.tensor_add(
                        out=pr, in0=p_tiles[2 * j], in1=p_tiles[2 * j + 1]
                    )
                    prs.append(pr)
                qds = []
                for j in range(2):
                    qd = tr_pool.tile([128, 1024], bf16, tag="tr", bufs=12, name=f"tq{h}_{j}")
                    nc.vector.tensor_add(out=qd, in0=prs[2 * j], in1=prs[2 * j + 1])
                    qds.append(qd)
                for tb in range(2):
                    sl = slice(tb * 512, (tb + 1) * 512)
                    z_ps = ps_pool.tile([1, 512], f32, tag="z", bufs=1,
                                        name=f"z{h}_{tb}")
                    for j in range(2):
                        nc.tensor.matmul(
                            z_ps, ones_colb, qds[j][:, sl],
                            start=(j == 0), stop=(j == 1),
                        )
                    z_sb = z_pool.tile([1, 512], f32r, tag="zsb", name=f"zsb{h}_{tb}")
                    nc.scalar.activation(out=z_sb, in_=z_ps, func=AF.Copy,
                                         bias=0.0, scale=1.0)
                    zr_ps = ps_pool.tile([128, 512], f32, tag="zr", bufs=1,
                                         name=f"zr{h}_{tb}")
                    nc.tensor.matmul(zr_ps, ones_k1, z_sb, start=True, stop=True)
                    av_ps = ps_pool.tile([128, 512], f32, tag="av", bufs=2,
                                         name=f"av{h}_{tb}")
                    for kb in range(8):
                        nc.tensor.matmul(
                            av_ps, v_sb[:, kb, h * 128 : (h + 1) * 128],
                            p_tiles[kb][:, sl],
                            start=(kb == 0), stop=(kb == 7),
                        )
                    zinv = z_pool.tile([128, 512], f32, tag="zinv", name=f"zi{h}_{tb}")
                    nc.vector.reciprocal_approx_fast(out=zinv, in_=zr_ps)
                    nc.vector.tensor_mul(
                        out=at_sb[:, h, sl], in0=av_ps, in1=zinv
                    )

            prev = None
            for h in range(G):
                expb_t = eb_pool.tile([128, 8, T], bf16, tag="eb", name=f"eb{h}")
                nc.sync.dma_start(
                    out=expb_t,
                    in_=ebT_d[h].rearrange("(kb p) t -> p kb t", p=128),
                )
                p_tiles = emit_scores(h, expb_t)
                if prev is not None:
                    emit_reduce(*prev)
                prev = (h, p_tiles)
            emit_reduce(*prev)

        # ---------------- Phase out-projection ----------------
        with (
            tc.tile_pool(name="wo", bufs=2) as wo_pool,
            tc.tile_pool(name="ot", bufs=4) as ot_pool,
            tc.tile_pool(name="pso", bufs=1, space="PSUM") as ps_pool,
        ):
            for eb in range(8):
                wo_t = wo_pool.tile([128, 8, 512], f16, tag="wo")
                nc.sync.dma_start(
                    out=wo_t,
                    in_=woT_d.rearrange("(ob p) e -> p ob e", p=128)[
                        :, :, eb * 512 : (eb + 1) * 512
                    ],
                )
                for tbh in range(2):
                    acc = [
                        ps_pool.tile([128, 512], f32, tag="out", bufs=8, name=f"acco{eb}_{tbh}_{j}")
                        for j in range(4)
                    ]
                    for ob in range(8):
                        for t4 in range(4):
                            toff = tbh * 512 + t4 * 128
                            nc.tensor.matmul(
                                acc[t4],
                                at_sb[:, ob, toff : toff + 128],
                                wo_t[:, ob, :],
                                start=(ob == 0),
                                stop=(ob == 7),
                            )
                    for t4 in range(4):
                        toff = tbh * 512 + t4 * 128
                        ot = ot_pool.tile([128, 512], f32, tag="ot")
                        if t4 % 2 == 0:
                            nc.vector.tensor_copy(out=ot, in_=acc[t4])
                        else:
                            nc.scalar.activation(
                                out=ot, in_=acc[t4], func=AF.Copy, bias=0.0, scale=1.0
                            )
                        nc.sync.dma_start(
                            out=out_d[toff : toff + 128, eb * 512 : (eb + 1) * 512],
                            in_=ot,
                        )

    nc.compile()
    return nc


def _get_program():
    global _PROGRAM
    if _PROGRAM is None:
        _PROGRAM = _build_program()
    return _PROGRAM


def kernel(hidden_states, key_value_states, attention_mask, position_bias,
           ln_weight, w_q, w_k, w_v, w_out):
    global LAST_RESULT
    from concourse.bass_utils import run_bass_kernel_spmd

    hidden_states = np.asarray(hidden_states, np.float32)
    key_value_states = np.asarray(key_value_states, np.float32)
    attention_mask = np.asarray(attention_mask)
    position_bias = np.asarray(position_bias, np.float32)
    ln_weight = np.asarray(ln_weight, np.float32)
    w_q = np.asarray(w_q, np.float32)
    w_k = np.asarray(w_k, np.float32)
    w_v = np.asarray(w_v, np.float32)
    w_out = np.asarray(w_out, np.float32)

    nc = _get_program()

    wq_eff = w_q * ln_weight[None, :]
    ones_c = np.ones((128, 1), np.float32)
    ones_k = np.ones((1, 128), np.float32)
    ones_cb = np.ones((128, 1), ml_dtypes.bfloat16)

    in_maps = []
    for c in range(8):
        b, g = divmod(c, 4)
        o0 = g * G * DH
        # expb[k, t] = mask[t, k] * exp(position_bias[t, k]) transposed
        pb_slice = position_bias[g * G : (g + 1) * G]          # [G, t, k]
        ebT = (
            np.where(attention_mask[b][None], np.exp(pb_slice), 0.0)
            .transpose(0, 2, 1)
        ).astype(ml_dtypes.bfloat16)                            # [G, k, t]
        in_maps.append({
            "xT": np.ascontiguousarray(hidden_states[b].T).astype(np.float16),
            "kvT": np.ascontiguousarray(key_value_states[b].T).astype(np.float16),
            "wqT": np.ascontiguousarray(wq_eff[o0 : o0 + G * DH].T).astype(np.float16),
            "wkT": np.ascontiguousarray(w_k[o0 : o0 + G * DH].T).astype(np.float16),
            "wvT": np.ascontiguousarray(w_v[o0 : o0 + G * DH].T).astype(np.float16),
            "woT": np.ascontiguousarray(w_out[:, o0 : o0 + G * DH].T).astype(np.float16),
            "ebT": np.ascontiguousarray(ebT),
            "onc": ones_c,
            "onk": ones_k,
            "oncb": ones_cb,
        })

    r = run_bass_kernel_spmd(nc, in_maps, core_ids=list(range(8)))
    LAST_RESULT = r

    res = np.empty((B, T, D), np.float32)
    for b in range(B):
        acc = hidden_states[b].copy()
        for g in range(4):
            acc += r.results[4 * b + g]["out"]
        res[b] = acc
    return res


# revision 15
# speedup vs baseline: 1.0165x; 1.0165x over previous
"""CPM3 cross-attention on 8 Trainium2 NeuronCores.

Sharding: 2-way data-parallel over batch x 4-way tensor-parallel over heads
(8 heads per core). Each core computes a partial output (its heads' slice of
the out-projection, pre-residual); the host sums the 4 partials per batch and
adds the residual.

Per-core math (fp16 operands on all projection/score matmuls, fp32 PSUM):
  rsqrt-scale of RMSNorm is applied to Q after the projection (linearity);
  ln_weight is folded into w_q on the host. Scores are computed transposed
  (S.T[k, t]) so no on-device transposes are needed. Softmax uses a fixed
  -60 offset instead of a row max (safe for this distribution) and the
  position bias + mask are applied MULTIPLICATIVELY after the exp:
  P = exp(S - 60) * expb where expb = mask * exp(position_bias) is
  precomputed on the host (bf16). Z = sum_k P via DVE pair-sums + short
  ones-matmul chains; normalization (1/Z) uses the fast approx reciprocal
  and is applied after P@V. V stays resident in SBUF (no DRAM spill).
"""

import numpy as np
import ml_dtypes
from contextlib import ExitStack

B = 2
T = 1024          # LQ == LK
D = 4096
H = 32
DH = 128
G = 8             # heads per core
ND = D // 128     # 32 contraction tiles
EPS = 1e-5
COFF = 60.0       # softmax fixed offset

_PROGRAM = None
LAST_RESULT = None


def _build_program():
    import concourse.mybir as mybir
    import concourse.tile as tile
    from concourse import bacc

    dt = mybir.dt
    f32 = dt.float32
    f32r = dt.float32r
    bf16 = dt.bfloat16
    f16 = dt.float16
    AF = mybir.ActivationFunctionType

    nc = bacc.Bacc(None, target_bir_lowering=False, dynamic_dma_scratch_size=2048)

    xT_d = nc.dram_tensor("xT", [D, T], f16, kind="ExternalInput")
    kvT_d = nc.dram_tensor("kvT", [D, T], f16, kind="ExternalInput")
    wqT_d = nc.dram_tensor("wqT", [D, G * DH], f16, kind="ExternalInput")
    wkT_d = nc.dram_tensor("wkT", [D, G * DH], f16, kind="ExternalInput")
    wvT_d = nc.dram_tensor("wvT", [D, G * DH], f16, kind="ExternalInput")
    woT_d = nc.dram_tensor("woT", [G * DH, D], f16, kind="ExternalInput")
    ebT_d = nc.dram_tensor("ebT", [G, T, T], bf16, kind="ExternalInput")
    onc_d = nc.dram_tensor("onc", [128, 1], f32r, kind="ExternalInput")
    onk_d = nc.dram_tensor("onk", [1, 128], f32r, kind="ExternalInput")
    oncb_d = nc.dram_tensor("oncb", [128, 1], bf16, kind="ExternalInput")
    out_d = nc.dram_tensor("out", [T, D], f32, kind="ExternalOutput")

    with (
        tile.TileContext(nc) as tc,
        ExitStack() as ctx,
        nc.allow_low_precision(reason="fp16/bf16 attention pipeline"),
    ):
        consts = ctx.enter_context(tc.tile_pool(name="consts", bufs=1))
        ones_k1 = consts.tile([1, 128], f32r)
        nc.sync.dma_start(out=ones_k1, in_=onk_d[:, :])
        ones_colb = consts.tile([128, 1], bf16)
        nc.sync.dma_start(out=ones_colb, in_=oncb_d[:, :])
        mbias = consts.tile([128, 1], f32)
        nc.vector.memset(mbias, -COFF)
        ebias = consts.tile([1, 1], f32)
        nc.vector.memset(ebias, EPS)

        kt_pool = ctx.enter_context(tc.tile_pool(name="kt", bufs=1))
        kt_sb = kt_pool.tile([128, G, T], f16)   # K.T: [dh(part), head, k]
        v_pool = ctx.enter_context(tc.tile_pool(name="v", bufs=1))
        v_sb = v_pool.tile([128, 8, G * DH], bf16)  # V: [k%128(part), kb, o]
        qt_pool = ctx.enter_context(tc.tile_pool(name="qt", bufs=1))
        qt_sb = qt_pool.tile([128, G, T], f16)  # Q.T: [dh(part), head, t]

        with tc.tile_pool(name="x", bufs=1) as x_pool:
            x_sb = x_pool.tile([128, ND, T], f16)

            # ---------------- Phase KV: K.T and V projections ----------------
            with (
                tc.tile_pool(name="kv", bufs=1) as kv_pool,
                tc.tile_pool(name="wkv", bufs=3) as w_pool,
                tc.tile_pool(name="pskv", bufs=1, space="PSUM") as ps_pool,
            ):
                kv_sb = kv_pool.tile([128, ND, T], f16)
                # first chunk split fine so the round-0 d=0 matmuls start ASAP
                for i in range(4):
                    nc.scalar.dma_start(
                        out=kv_sb[:, i : i + 1, :],
                        in_=kvT_d.rearrange("(n p) t -> p n t", p=128)[
                            :, i : i + 1, :
                        ],
                    )
                for i in range(1, 8):
                    nc.scalar.dma_start(
                        out=kv_sb[:, i * 4 : (i + 1) * 4, :],
                        in_=kvT_d.rearrange("(n p) t -> p n t", p=128)[
                            :, i * 4 : (i + 1) * 4, :
                        ],
                    )
                # K.T: 2 rounds over o-halves, 8 psum banks (DMA-paced round 0)
                for r in range(2):
                    if r == 1:
                        # prefetch x for the Q phase; issued after round-0 DMAs
                        # so the critical kv/wk transfers get the early bandwidth
                        for i in range(8):
                            nc.scalar.dma_start(
                                out=x_sb[:, i * 4 : (i + 1) * 4, :],
                                in_=xT_d.rearrange("(n p) t -> p n t", p=128)[
                                    :, i * 4 : (i + 1) * 4, :
                                ],
                            )
                    acc = [
                        ps_pool.tile([128, 512], f32, tag="acc", bufs=8, name=f"acck{r}_{j}")
                        for j in range(8)
                    ]
                    for dg in range(ND // 4):
                        wk_t = w_pool.tile([128, 4, 512], f16, tag="wk")
                        nc.sync.dma_start(
                            out=wk_t,
                            in_=wkT_d.rearrange("(n p) o -> p n o", p=128)[
                                :, dg * 4 : (dg + 1) * 4, r * 512 : (r + 1) * 512
                            ],
                        )
                        for dd in range(4):
                            d = dg * 4 + dd
                            for ob2 in range(4):
                                for tkb in range(2):
                                    nc.tensor.matmul(
                                        acc[ob2 * 2 + tkb],
                                        wk_t[:, dd, ob2 * 128 : (ob2 + 1) * 128],
                                        kv_sb[:, d, tkb * 512 : (tkb + 1) * 512],
                                        start=(d == 0),
                                        stop=(d == ND - 1),
                                    )
                    for ob2 in range(4):
                        for tkb in range(2):
                            ob = r * 4 + ob2
                            dst = kt_sb[:, ob, tkb * 512 : (tkb + 1) * 512]
                            src = acc[ob2 * 2 + tkb]
                            if tkb == 0:
                                nc.vector.tensor_copy(out=dst, in_=src)
                            else:
                                nc.scalar.activation(
                                    out=dst, in_=src, func=AF.Copy, bias=0.0, scale=1.0
                                )

                # V (fp16 matmuls from resident kv, resident bf16 output)
                for r in range(2):
                    acc8 = [
                        ps_pool.tile([128, 512], f32, tag="acc", bufs=8, name=f"accv{r}_{j}")
                        for j in range(8)
                    ]
                    for dg in range(ND // 4):
                        wv_t = w_pool.tile([128, 4, 512], f16, tag="wv")
                        nc.sync.dma_start(
                            out=wv_t,
                            in_=wvT_d.rearrange("(n p) o -> p n o", p=128)[
                                :, dg * 4 : (dg + 1) * 4, r * 512 : (r + 1) * 512
                            ],
                        )
                        for dd in range(4):
                            d = dg * 4 + dd
                            for tkb in range(8):
                                nc.tensor.matmul(
                                    acc8[tkb],
                                    kv_sb[:, d, tkb * 128 : (tkb + 1) * 128],
                                    wv_t[:, dd, :],
                                    start=(d == 0),
                                    stop=(d == ND - 1),
                                )
                    for tkb in range(8):
                        dst = v_sb[:, tkb, r * 512 : (r + 1) * 512]
                        if tkb % 2 == 0:
                            nc.vector.tensor_copy(out=dst, in_=acc8[tkb])
                        else:
                            nc.scalar.activation(
                                out=dst, in_=acc8[tkb], func=AF.Copy, bias=0.0, scale=1.0
                            )

            # ---------------- Phase Q: rsqrt stats + Q projection ----------------
            with (
                tc.tile_pool(name="wq", bufs=3) as wq_pool,
                tc.tile_pool(name="sq", bufs=2) as sq_pool,
                tc.tile_pool(name="srow", bufs=1) as srow_pool,
                tc.tile_pool(name="psq", bufs=1, space="PSUM") as ps_pool,
            ):
                # var[t] = sum_d x^2 (ACT Square + ones-matmul), interleaved with
                # Q-proj round 0.
                var_ps = [
                    ps_pool.tile([1, 512], f32, tag="var", bufs=2, name=f"var{j}")
                    for j in range(2)
                ]
                acc0 = [
                    ps_pool.tile([128, 512], f32, tag="acc", bufs=6, name=f"accq0_{j}")
                    for j in range(4)
                ]
                for dg in range(ND // 4):
                    wq_t = wq_pool.tile([128, 4, 256], f16, tag="wqt")
                    nc.sync.dma_start(
                        out=wq_t,
                        in_=wqT_d.rearrange("(n p) o -> p n o", p=128)[
                            :, dg * 4 : (dg + 1) * 4, 0:256
                        ],
                    )
                    sqs = [[], []]
                    for dd in range(4):
                        d = dg * 4 + dd
                        for tb in range(2):
                            sq = sq_pool.tile([128, 512], bf16, tag="sq", bufs=12)
                            nc.scalar.activation(
                                out=sq,
                                in_=x_sb[:, d, tb * 512 : (tb + 1) * 512],
                                func=AF.Square,
                            )
                            sqs[tb].append(sq)
                        for ob2 in range(2):
                            for tb in range(2):
                                nc.tensor.matmul(
                                    acc0[ob2 * 2 + tb],
                                    wq_t[:, dd, ob2 * 128 : (ob2 + 1) * 128],
                                    x_sb[:, d, tb * 512 : (tb + 1) * 512],
                                    start=(d == 0),
                                    stop=(d == ND - 1),
                                )
                    # quad-sum the squares on DVE so var costs 1 matmul per dg
                    for tb in range(2):
                        qa = sq_pool.tile([128, 512], bf16, tag="sqa", bufs=8)
                        nc.vector.tensor_add(out=qa, in0=sqs[tb][0], in1=sqs[tb][1])
                        qb = sq_pool.tile([128, 512], bf16, tag="sqa", bufs=8)
                        nc.vector.tensor_add(out=qb, in0=sqs[tb][2], in1=sqs[tb][3])
                        qd = sq_pool.tile([128, 512], bf16, tag="sqa", bufs=8)
                        nc.vector.tensor_add(out=qd, in0=qa, in1=qb)
                        nc.tensor.matmul(
                            var_ps[tb], ones_colb, qd,
                            start=(dg == 0), stop=(dg == ND // 4 - 1),
                        )
                # s = 1/sqrt(var/D + eps), replicated to 128 partitions
                # replicate sqrt(var/D+eps) to 128 partitions, then take the
                # approx reciprocal wide (128 lanes) instead of on one row
                sd_row = srow_pool.tile([1, T], f32r)
                for tb in range(2):
                    nc.scalar.activation(
                        out=sd_row[:, tb * 512 : (tb + 1) * 512],
                        in_=var_ps[tb],
                        func=AF.Sqrt,
                        bias=ebias,
                        scale=1.0 / D,
                    )
                s_rep = srow_pool.tile([128, T], f32)
                for tb in range(2):
                    srp = ps_pool.tile([128, 512], f32, tag="var", bufs=2, name=f"srp{tb}")
                    nc.tensor.matmul(
                        srp, ones_k1,
                        sd_row[:, tb * 512 : (tb + 1) * 512],
                        start=True, stop=True,
                    )
                    nc.vector.reciprocal_approx_fast(
                        out=s_rep[:, tb * 512 : (tb + 1) * 512], in_=srp
                    )

                # Q.T round-0 writes (scale folded), then rounds 1-3
                for ob2 in range(2):
                    for tb in range(2):
                        nc.vector.tensor_mul(
                            out=qt_sb[:, ob2, tb * 512 : (tb + 1) * 512],
                            in0=acc0[ob2 * 2 + tb],
                            in1=s_rep[:, tb * 512 : (tb + 1) * 512],
                        )
                for r in range(1, 4):
                    acc = [
                        ps_pool.tile([128, 512], f32, tag="acc", bufs=6, name=f"accq{r}_{j}")
                        for j in range(4)
                    ]
                    for dg in range(ND // 4):
                        wq_t = wq_pool.tile([128, 4, 256], f16, tag="wqt")
                        nc.sync.dma_start(
                            out=wq_t,
                            in_=wqT_d.rearrange("(n p) o -> p n o", p=128)[
                                :, dg * 4 : (dg + 1) * 4, r * 256 : (r + 1) * 256
                            ],
                        )
                        for dd in range(4):
                            d = dg * 4 + dd
                            for ob2 in range(2):
                                for tb in range(2):
                                    nc.tensor.matmul(
                                        acc[ob2 * 2 + tb],
                                        wq_t[:, dd, ob2 * 128 : (ob2 + 1) * 128],
                                        x_sb[:, d, tb * 512 : (tb + 1) * 512],
                                        start=(d == 0),
                                        stop=(d == ND - 1),
                                    )
                    for ob2 in range(2):
                        for tb in range(2):
                            ob = r * 2 + ob2
                            nc.vector.tensor_mul(
                                out=qt_sb[:, ob, tb * 512 : (tb + 1) * 512],
                                in0=acc[ob2 * 2 + tb],
                                in1=s_rep[:, tb * 512 : (tb + 1) * 512],
                            )

        at_pool = ctx.enter_context(tc.tile_pool(name="at", bufs=1))
        at_sb = at_pool.tile([128, G, T], f16)  # A_all.T: [dh(part), head, t]

        # ---------------- Phase attention (per head) ----------------
        with (
            tc.tile_pool(name="ebp", bufs=3) as eb_pool,
            tc.tile_pool(name="esp", bufs=6) as es_pool,
            tc.tile_pool(name="pp", bufs=20) as p_pool,
            tc.tile_pool(name="trp", bufs=8) as tr_pool,
            tc.tile_pool(name="zrow", bufs=4) as z_pool,
            tc.tile_pool(name="psat", bufs=1, space="PSUM") as ps_pool,
        ):
            def emit_scores(h, expb_t):
                """Scores + exp + bias-multiply for head h; P tiles [128, 1024]."""
                p_tiles = []
                for kb in range(8):
                    sps = ps_pool.tile([128, 1024], f32, tag="sc", bufs=2,
                                       name=f"sc{h}_{kb}")
                    for tb in range(2):
                        nc.tensor.matmul(
                            sps[:, tb * 512 : (tb + 1) * 512],
                            kt_sb[:, h, kb * 128 : (kb + 1) * 128],
                            qt_sb[:, h, tb * 512 : (tb + 1) * 512],
                            start=True,
                            stop=True,
                        )
                    es = es_pool.tile([128, 1024], bf16, tag="es", name=f"es{h}_{kb}")
                    nc.scalar.activation(
                        out=es, in_=sps, func=AF.Exp, bias=mbias, scale=1.0
                    )
                    p_t = p_pool.tile([128, 1024], bf16, tag="p", name=f"p{h}_{kb}")
                    nc.vector.tensor_mul(out=p_t, in0=es, in1=expb_t[:, kb, :])
                    p_tiles.append(p_t)
                return p_tiles

            def emit_reduce(h, p_tiles):
                """Z, AV, normalize for head h."""
                prs = []
                for j in range(4):
                    pr = tr_pool.tile([128, 1024], bf16, tag="tr", bufs=12, name=f"tr{h}_{j}")
                    nc.vector.tensor_add(
                        out=pr, in0=p_tiles[2 * j], in1=p_tiles[2 * j + 1]
                    )
                    prs.append(pr)
                for tb in range(2):
                    sl = slice(tb * 512, (tb + 1) * 512)
                    z_ps = ps_pool.tile([1, 512], f32, tag="z", bufs=1,
                                        name=f"z{h}_{tb}")
                    for j in range(4):
                        nc.tensor.matmul(
                            z_ps, ones_colb, prs[j][:, sl],
                            start=(j == 0), stop=(j == 3),
                        )
                    z_sb = z_pool.tile([1, 512], f32r, tag="zsb", name=f"zsb{h}_{tb}")
                    nc.scalar.activation(out=z_sb, in_=z_ps, func=AF.Copy,
                                         bias=0.0, scale=1.0)
                    zr_ps = ps_pool.tile([128, 512], f32, tag="zr", bufs=1,
                                         name=f"zr{h}_{tb}")
                    nc.tensor.matmul(zr_ps, ones_k1, z_sb, start=True, stop=True)
                    av_ps = ps_pool.tile([128, 512], f32, tag="av", bufs=2,
                                         name=f"av{h}_{tb}")
                    for kb in range(8):
                        nc.tensor.matmul(
                            av_ps, v_sb[:, kb, h * 128 : (h + 1) * 128],
                            p_tiles[kb][:, sl],
                            start=(kb == 0), stop=(kb == 7),
                        )
                    zinv = z_pool.tile([128, 512], f32, tag="zinv", name=f"zi{h}_{tb}")
                    nc.vector.reciprocal_approx_fast(out=zinv, in_=zr_ps)
                    nc.vector.tensor_mul(
                        out=at_sb[:, h, sl], in0=av_ps, in1=zinv
                    )

            prev = None
            for h in range(G):
                expb_t = eb_pool.tile([128, 8, T], bf16, tag="eb", name=f"eb{h}")
                nc.sync.dma_start(
                    out=expb_t,
                    in_=ebT_d[h].rearrange("(kb p) t -> p kb t", p=128),
                )
                p_tiles = emit_scores(h, expb_t)
                if prev is not None:
                    emit_reduce(*prev)
                prev = (h, p_tiles)
            emit_reduce(*prev)

        # ---------------- Phase out-projection ----------------
        with (
            tc.tile_pool(name="wo", bufs=2) as wo_pool,
            tc.tile_pool(name="ot", bufs=4) as ot_pool,
            tc.tile_pool(name="pso", bufs=1, space="PSUM") as ps_pool,
        ):
            for eb in range(8):
                wo_t = wo_pool.tile([128, 8, 512], f16, tag="wo")
                nc.sync.dma_start(
                    out=wo_t,
                    in_=woT_d.rearrange("(ob p) e -> p ob e", p=128)[
                        :, :, eb * 512 : (eb + 1) * 512
                    ],
                )
                for tbh in range(2):
                    acc = [
                        ps_pool.tile([128, 512], f32, tag="out", bufs=8, name=f"acco{eb}_{tbh}_{j}")
                        for j in range(4)
                    ]
                    for ob in range(8):
                        for t4 in range(4):
                            toff = tbh * 512 + t4 * 128
                            nc.tensor.matmul(
                                acc[t4],
                                at_sb[:, ob, toff : toff + 128],
                                wo_t[:, ob, :],
                                start=(ob == 0),
                                stop=(ob == 7),
                            )
                    for t4 in range(4):
                        toff = tbh * 512 + t4 * 128
                        ot = ot_pool.tile([128, 512], f32, tag="ot")
                        if t4 % 2 == 0:
                            nc.vector.tensor_copy(out=ot, in_=acc[t4])
                        else:
                            nc.scalar.activation(
                                out=ot, in_=acc[t4], func=AF.Copy, bias=0.0, scale=1.0
                            )
                        nc.sync.dma_start(
                            out=out_d[toff : toff + 128, eb * 512 : (eb + 1) * 512],
                            in_=ot,
                        )

    nc.compile()
    return nc


def _get_program():
    global _PROGRAM
    if _PROGRAM is None:
        _PROGRAM = _build_program()
    return _PROGRAM


def kernel(hidden_states, key_value_states, attention_mask, position_bias,
           ln_weight, w_q, w_k, w_v, w_out):
    global LAST_RESULT
    from concourse.bass_utils import run_bass_kernel_spmd

    hidden_states = np.asarray(hidden_states, np.float32)
    key_value_states = np.asarray(key_value_states, np.float32)
    attention_mask = np.asarray(attention_mask)
    position_bias = np.asarray(position_bias, np.float32)
    ln_weight = np.asarray(ln_weight, np.float32)
    w_q = np.asarray(w_q, np.float32)
    w_k = np.asarray(w_k, np.float32)
    w_v = np.asarray(w_v, np.float32)
    w_out = np.asarray(w_out, np.float32)

    nc = _get_program()

    wq_eff = w_q * ln_weight[None, :]
    ones_c = np.ones((128, 1), np.float32)
    ones_k = np.ones((1, 128), np.float32)
    ones_cb = np.ones((128, 1), ml_dtypes.bfloat16)

    in_maps = []
    for c in range(8):
        b, g = divmod(c, 4)
        o0 = g * G * DH
        # expb[k, t] = mask[t, k] * exp(position_bias[t, k]) transposed
        pb_slice = position_bias[g * G : (g + 1) * G]          # [G, t, k]
        ebT = (
            np.where(attention_mask[b][None], np.exp(pb_slice), 0.0)
            .transpose(0, 2, 1)
        ).astype(ml_dtypes.bfloat16)                            # [G, k, t]
        in_maps.append({
            "xT": np.ascontiguousarray(hidden_states[b].T).astype(np.float16),
            "kvT": np.ascontiguousarray(key_value_states[b].T).astype(np.float16),
            "wqT": np.ascontiguousarray(wq_eff[o0 : o0 + G * DH].T).astype(np.float16),
            "wkT": np.ascontiguousarray(w_k[o0 : o0 + G * DH].T).astype(np.float16),
            "wvT": np.ascontiguousarray(w_v[o0 : o0 + G * DH].T).astype(np.float16),
            "woT": np.ascontiguousarray(w_out[:, o0 : o0 + G * DH].T).astype(np.float16),
            "ebT": np.ascontiguousarray(ebT),
            "onc": ones_c,
            "onk": ones_k,
            "oncb": ones_cb,
        })

    r = run_bass_kernel_spmd(nc, in_maps, core_ids=list(range(8)))
    LAST_RESULT = r

    res = np.empty((B, T, D), np.float32)
    for b in range(B):
        acc = hidden_states[b].copy()
        for g in range(4):
            acc += r.results[4 * b + g]["out"]
        res[b] = acc
    return res
